# revision 10
# baseline (speedup 1.0000x reference)
"""Trainium2 Bass kernel for grouped block-diagonal MLP (gnn_message_passing).

Computation: out[b, 3g+j] = sum_i x[b, 15g+i] * W[g, j, i]   (g<25, i<15, j<3)
Equivalent to out = x @ Wd where Wd is a [375, 75] block-diagonal matrix built
from the 25 stacked [3, 15] Linear weights (scattered per k_idx/v_idx).

Strategy (pure data parallel, 8 cores), v5:
  - memory-regime problem: halve HBM traffic with bf16 (harness gate is 2e-2,
    bf16 end-to-end lands ~3e-3) and remove every on-device transpose by
    staging x TRANSPOSED on the host, laid out so each input DMA reads fully
    contiguous multi-KB runs per partition. The 375 contraction rows split as
    two tensors to avoid padding traffic: xa [128, 8, 2, 4096] (K-chunks 0,1)
    and xb [119, 8, 4096] (K-chunk 2).
  - per core: out.T[75, B/8] = sum_c Wd_c.T @ xT_c with the Wd chunk as the PE
    stationary operand (75-col LDWEIGHTS) and xT streaming as the moving
    operand in 512-col sub-blocks, accumulating the 3 K-chunks in PSUM.
    DVE + ACT casts move each group fp32 PSUM -> bf16 SBUF in parallel halves.
  - input DMAs ride the sync (SP) HWDGE ring; weight + output DMAs ride the
    scalar (ACT) HWDGE ring so writes never FIFO-serialize behind the input
    stream. Work is cut in tapered pieces (7x4096, 2048, 1024, 512, 512) so
    the end-of-stream tail only waits on tiny transfers/bursts. Output goes
    back transposed ([75, B/8] bf16) and is un-transposed on the host.
"""

import numpy as np
import ml_dtypes

BF16 = np.dtype(ml_dtypes.bfloat16)

B = 262144
NCORES = 8
B_CORE = B // NCORES  # 32768
F = 375   # input cols (25 groups * 15)
KC2 = F - 256  # 119 rows in K-chunk 2
O = 75    # output cols (25 groups * 3)
OUT_DIM = 75
NB = 4096          # batch cols per full piece (one input DMA)
N_SUP = B_CORE // NB  # 8
NSB = 512          # moving-operand free size per matmul
NG = 2048          # batch cols per PSUM group (4 banks)

_compiled = {}


def _pieces():
    ps = [(s, 0, NB) for s in range(N_SUP - 1)]
    ps += [
        (N_SUP - 1, 0, 2048),
        (N_SUP - 1, 2048, 1024),
        (N_SUP - 1, 3072, 512),
        (N_SUP - 1, 3584, 512),
    ]
    return ps


def _build_bass():
    import concourse.mybir as mybir
    import concourse.tile as tile
    from concourse import bacc

    f32 = mybir.dt.float32
    bf16 = mybir.dt.bfloat16
    nc = bacc.Bacc()
    xa_d = nc.dram_tensor("xa", [128, N_SUP, 2, NB], bf16, kind="ExternalInput")
    xb_d = nc.dram_tensor("xb", [KC2, N_SUP, NB], bf16, kind="ExternalInput")
    w_d = nc.dram_tensor("wd", [3, 128, O], bf16, kind="ExternalInput")
    ot_d = nc.dram_tensor("ot", [O, B_CORE], bf16, kind="ExternalOutput")

    with tile.TileContext(nc) as tc:
        with (
            tc.tile_pool(name="const", bufs=1) as cpool,
            tc.tile_pool(name="xina", bufs=5) as xapool,
            tc.tile_pool(name="xinb", bufs=5) as xbpool,
            tc.tile_pool(name="osb", bufs=4) as opool,
            tc.tile_pool(name="acc", bufs=2, space="PSUM") as pacc,
        ):
            wd = cpool.tile([128, 3, O], bf16)
            nc.scalar.dma_start(wd[:], w_d[:].rearrange("c k n -> k c n"))

            # PE instructions carry at most one semaphore wait; burn the wd
            # DMA dep with a throwaway matmul so real matmuls only wait on
            # their x DMA.
            warm = pacc.tile([128, NG], f32, tag="acc")
            nc.tensor.matmul(
                warm[:O, :O], wd[:, 0, :], wd[:, 0, :], start=True, stop=True
            )

            for s, n0, nb in _pieces():
                r0 = s * NB + n0
                xina = xapool.tile([128, 2, nb], bf16, tag="xina")
                nc.sync.dma_start(xina[:], xa_d[:, s, :, n0 : n0 + nb])
                xinb = xbpool.tile([KC2, nb], bf16, tag="xinb")
                nc.sync.dma_start(xinb[:], xb_d[:, s, n0 : n0 + nb])
                g0 = 0
                while g0 < nb:
                    gs = min(NG, nb - g0)
                    acc = pacc.tile([128, gs], f32, tag="acc")
                    for c in range(3):
                        for sb in range(gs // NSB):
                            col0 = g0 + sb * NSB
                            if c < 2:
                                rhs = xina[:, c, col0 : col0 + NSB]
                                lhsT = wd[:, c, :]
                            else:
                                rhs = xinb[:, col0 : col0 + NSB]
                                lhsT = wd[:KC2, c, :]
                            nc.tensor.matmul(
                                acc[:O, sb * NSB : (sb + 1) * NSB],
                                lhsT,
                                rhs,
                                start=(c == 0),
                                stop=(c == 2),
                            )
                    osb = opool.tile([O, gs], bf16, tag="osb")
                    if gs >= NG:
                        half = gs // 2
                        nc.vector.tensor_copy(osb[:, :half], acc[:O, :half])
                        nc.scalar.copy(osb[:, half:], acc[:O, half:])
                    else:
                        nc.vector.tensor_copy(osb[:], acc[:O, :gs])
                    nc.scalar.dma_start(ot_d[:, r0 + g0 : r0 + g0 + gs], osb[:])
                    g0 += gs
    nc.compile()
    return nc


def _get_nc():
    if "nc" not in _compiled:
        _compiled["nc"] = _build_bass()
    return _compiled["nc"]


def _build_wd_chunks(W, k_idx, v_idx):
    """Dense [3, 128, 75] chunked block-diagonal weight from stacked W.
    Chunk 2 rows >= KC2 are zero (never multiplied against real data)."""
    Wd = np.zeros((384, O), dtype=np.float32)
    kk = np.asarray(k_idx)
    vv = np.asarray(v_idx)
    Ww = np.asarray(W)
    # Wd[k_idx[g,i], v_idx[g,j]] = W[g, j, i]
    Wd[kk[:, :, None], vv[:, None, :]] = Ww.transpose(0, 2, 1)
    return np.ascontiguousarray(Wd.reshape(3, 128, O).astype(BF16))


def _shard_x(x, i):
    """Core i's inputs: xa [128, N_SUP, 2, NB] (K rows 0..255) and
    xb [119, N_SUP, NB] (K rows 256..374), both bf16 with
    x[i*B_CORE + s*NB + n, k] at [k%128 or k-256, s, (k//128), n]."""
    xT = x[i * B_CORE : (i + 1) * B_CORE].T.astype(BF16)  # [375, B_CORE]
    xa = np.ascontiguousarray(
        xT[:256].reshape(2, 128, N_SUP, NB).transpose(1, 2, 0, 3)
    )
    xb = np.ascontiguousarray(xT[256:].reshape(KC2, N_SUP, NB))
    return xa, xb


def kernel(x, W, k_idx, v_idx, **_unused):
    from concourse.bass_utils import run_bass_kernel_spmd

    x = np.asarray(x, dtype=np.float32)
    wd3 = _build_wd_chunks(W, k_idx, v_idx)
    nc = _get_nc()

    in_maps = []
    for i in range(NCORES):
        xa, xb = _shard_x(x, i)
        in_maps.append({"xa": xa, "xb": xb, "wd": wd3})
    res = run_bass_kernel_spmd(nc, in_maps, list(range(NCORES)))
    parts = [res.results[i]["ot"] for i in range(NCORES)]
    got = np.concatenate(parts, axis=1).T.astype(np.float32)  # [B, 75]

    vflat = np.asarray(v_idx).reshape(-1)
    if vflat.shape[0] == OUT_DIM and np.array_equal(vflat, np.arange(OUT_DIM)):
        return np.ascontiguousarray(got)
    out = np.zeros((x.shape[0], OUT_DIM), dtype=np.float32)
    out[:, vflat] = got
    return out


# revision 11
# speedup vs baseline: 1.2620x; 1.2620x over previous
"""Trainium2 Bass kernel for grouped block-diagonal MLP (gnn_message_passing).

Computation: out[b, 3g+j] = sum_i x[b, 15g+i] * W[g, j, i]   (g<25, i<15, j<3)
Equivalent to out = x @ Wd where Wd is a [375, 75] block-diagonal matrix built
from the 25 stacked [3, 15] Linear weights (scattered per k_idx/v_idx).

Strategy (pure data parallel, 8 cores), v6:
  - memory-regime problem: halve HBM traffic with bf16 (harness gate is 2e-2,
    bf16 end-to-end lands ~3e-3) and remove every on-device transpose by
    staging x TRANSPOSED on the host, laid out so each input DMA is one fully
    contiguous 24 KB run per partition: xt [128, 8 supers, 3 K-chunks, 4096]
    bf16 per core (K rows 375..383 zero-padded for a uniform K=128; a split
    375-row layout was tried and badly imbalances the SDMA engines).
  - per core: out.T[75, B/8] = sum_c Wd_c.T @ xT_c with the Wd chunk as the PE
    stationary operand (75-col LDWEIGHTS) and xT streaming as the moving
    operand in 512-col sub-blocks, accumulating the 3 K-chunks in PSUM
    (up to 4 banks per group, 2 groups in flight; xin 5-deep so the DMA
    stream never stalls on HAM-cold PE bursts). DVE + ACT casts move each
    group fp32 PSUM -> bf16 SBUF in parallel halves.
  - input DMAs ride the sync (SP) HWDGE ring; weight + output DMAs ride the
    scalar (ACT) HWDGE ring so writes never FIFO-serialize behind the input
    stream. Work is cut in tapered pieces (7x4096, 2048, 1024, 512, 512) so
    the end-of-stream tail only waits on tiny transfers/bursts. Output goes
    back transposed ([75, B/8] bf16) and is un-transposed on the host.
"""

import numpy as np
import ml_dtypes

BF16 = np.dtype(ml_dtypes.bfloat16)

B = 262144
NCORES = 8
B_CORE = B // NCORES  # 32768
F = 375   # input cols (25 groups * 15)
FP = 384  # padded to 3 chunks of 128
O = 75    # output cols (25 groups * 3)
OUT_DIM = 75
NB = 4096          # batch cols per full piece (one input DMA)
N_SUP = B_CORE // NB  # 8
NSB = 512          # moving-operand free size per matmul
NG = 2048          # batch cols per PSUM group (4 banks)

_compiled = {}


def _pieces():
    ps = [(s, 0, NB) for s in range(N_SUP - 1)]
    ps += [
        (N_SUP - 1, 0, 2048),
        (N_SUP - 1, 2048, 1024),
        (N_SUP - 1, 3072, 512),
        (N_SUP - 1, 3584, 512),
    ]
    return ps


def _build_bass():
    import concourse.mybir as mybir
    import concourse.tile as tile
    from concourse import bacc

    f32 = mybir.dt.float32
    bf16 = mybir.dt.bfloat16
    nc = bacc.Bacc()
    xt_d = nc.dram_tensor("xt", [128, N_SUP, 3, NB], bf16, kind="ExternalInput")
    w_d = nc.dram_tensor("wd", [3, 128, O], bf16, kind="ExternalInput")
    ot_d = nc.dram_tensor("ot", [O, B_CORE], bf16, kind="ExternalOutput")

    with tile.TileContext(nc) as tc:
        with (
            tc.tile_pool(name="const", bufs=1) as cpool,
            tc.tile_pool(name="xin", bufs=5) as xpool,
            tc.tile_pool(name="osb", bufs=4) as opool,
            tc.tile_pool(name="acc", bufs=2, space="PSUM") as pacc,
        ):
            wd = cpool.tile([128, 3, O], bf16)
            nc.scalar.dma_start(wd[:], w_d[:].rearrange("c k n -> k c n"))

            # PE instructions carry at most one semaphore wait; burn the wd
            # DMA dep with a throwaway matmul so real matmuls only wait on
            # their x DMA.
            warm = pacc.tile([128, NG], f32, tag="acc")
            nc.tensor.matmul(
                warm[:O, :O], wd[:, 0, :], wd[:, 0, :], start=True, stop=True
            )

            for s, n0, nb in _pieces():
                r0 = s * NB + n0
                xin = xpool.tile([128, 3, nb], bf16, tag="xin")
                nc.sync.dma_start(xin[:], xt_d[:, s, :, n0 : n0 + nb])
                g0 = 0
                while g0 < nb:
                    gs = min(NG, nb - g0)
                    acc = pacc.tile([128, gs], f32, tag="acc")
                    for c in range(3):
                        for sb in range(gs // NSB):
                            col0 = g0 + sb * NSB
                            nc.tensor.matmul(
                                acc[:O, sb * NSB : (sb + 1) * NSB],
                                wd[:, c, :],
                                xin[:, c, col0 : col0 + NSB],
                                start=(c == 0),
                                stop=(c == 2),
                            )
                    osb = opool.tile([O, gs], bf16, tag="osb")
                    if gs >= NG:
                        half = gs // 2
                        nc.vector.tensor_copy(osb[:, :half], acc[:O, :half])
                        nc.scalar.copy(osb[:, half:], acc[:O, half:])
                    else:
                        nc.vector.tensor_copy(osb[:], acc[:O, :gs])
                    nc.scalar.dma_start(ot_d[:, r0 + g0 : r0 + g0 + gs], osb[:])
                    g0 += gs
    nc.compile()
    return nc


def _get_nc():
    if "nc" not in _compiled:
        _compiled["nc"] = _build_bass()
    return _compiled["nc"]


def _build_wd_chunks(W, k_idx, v_idx):
    """Dense [3, 128, 75] chunked block-diagonal weight from stacked W."""
    Wd = np.zeros((FP, O), dtype=np.float32)
    kk = np.asarray(k_idx)
    vv = np.asarray(v_idx)
    Ww = np.asarray(W)
    # Wd[k_idx[g,i], v_idx[g,j]] = W[g, j, i]
    Wd[kk[:, :, None], vv[:, None, :]] = Ww.transpose(0, 2, 1)
    return np.ascontiguousarray(Wd.reshape(3, 128, O).astype(BF16))


def _shard_x(x, i):
    """Core i's input: [128, N_SUP, 3, NB] bf16 with xt[p,s,c,n] =
    x[i*B_CORE + s*NB + n, c*128 + p] (rows >= F are zero padding)."""
    xT = np.zeros((FP, B_CORE), dtype=BF16)
    xT[:F] = x[i * B_CORE : (i + 1) * B_CORE].T.astype(BF16)
    return np.ascontiguousarray(
        xT.reshape(3, 128, N_SUP, NB).transpose(1, 2, 0, 3)
    )


def kernel(x, W, k_idx, v_idx, **_unused):
    from concourse.bass_utils import run_bass_kernel_spmd

    x = np.asarray(x, dtype=np.float32)
    wd3 = _build_wd_chunks(W, k_idx, v_idx)
    nc = _get_nc()

    in_maps = [{"xt": _shard_x(x, i), "wd": wd3} for i in range(NCORES)]
    res = run_bass_kernel_spmd(nc, in_maps, list(range(NCORES)))
    parts = [res.results[i]["ot"] for i in range(NCORES)]
    got = np.concatenate(parts, axis=1).T.astype(np.float32)  # [B, 75]

    vflat = np.asarray(v_idx).reshape(-1)
    if vflat.shape[0] == OUT_DIM and np.array_equal(vflat, np.arange(OUT_DIM)):
        return np.ascontiguousarray(got)
    out = np.zeros((x.shape[0], OUT_DIM), dtype=np.float32)
    out[:, vflat] = got
    return out
